# revision 41
# baseline (speedup 1.0000x reference)
"""Distributed Trainium2 Bass kernel for single-head attention with
softmax over the QUERY axis (faithful to the reference).

Reference math (per batch b):
    q = x @ Wq + bq          # [S, D]   S=4096, D=48
    k = x @ Wk + bk
    v = x @ Wv + bv
    s = (q @ k.T) / sqrt(D)  # [S_q, S_k]
    p = softmax(s, axis=QUERY)          # normalize each k-COLUMN over q
    out = p @ v              # [S_q, D]

Sharding: 8 cores = 4 batches x 2 query-halves. Core c handles batch
c//2, query rows [ (c%2)*2048, (c%2+1)*2048 ).

Layout: everything is computed TRANSPOSED on chip; scores_T[k, q] tiles
have k on partitions / q on the free axis so the softmax denominator
colsum[k] = sum_q exp(s[q,k]) falls out of the ScalarEngine's
activation accumulator, and the normalization folds into V. Output is
out_T [48, 2048], transposed on host.

Structure (the kernel is ScalarEngine/exp-bound; collectives on this
stack are high-variance, so the design minimizes exposure to them):
  - PE: scores matmuls 2x ROW-packed (K^T/Q^T duplicated at partitions
    64-111 via a 112-col packed Wq and an SBUF->SBUF DMA; two k-tiles
    stream concurrently in disjoint row-quadrants), attention matmuls
    2x COLUMN-packed by qc parity into a persistent 2-bank PSUM
    accumulator, projections in 1024-wide chunks woven as small filler
    pieces so the exp stream never starves.
  - Scores run in h-major sweeps per k-tile group, so the second Q
    chunk's projection hides inside the first sweep.
  - Colsum AllReduces: a dummy AR at t=0 absorbs stream setup; groups
    [0:8/8:16/16:24/24:28] AR mid-phase while exps continue. The LAST
    group (k-tiles 28-31) needs NO collective: both pair cores compute
    those 512 columns' partner-half exp sums redundantly (the full x
    is resident; +8 scratch ACTIVATEs) so its colsum is local and its
    attention + the output epilogue form a short deterministic tail.
    The redundant sweep also gives the group-3 AR ~25us of slack.

exp() runs without max-subtraction: scores*scale is N(0,~1/9), bounded
by ~|2.5| for these inputs, so exp stays well inside fp32 range
(softmax is shift-invariant, so the result matches the reference).
"""

import sys

for _p in ("/opt/trn_rl_repo",):
    if _p not in sys.path:
        sys.path.insert(0, _p)

import numpy as np
import ml_dtypes

import concourse.bass as bass
import concourse.tile as tile
from concourse import bacc, mybir
from concourse.bass_utils import run_bass_kernel_spmd
from concourse.masks import make_identity

N_CORES = 8
B = 4
S = 4096
DIM = 768
D = 48
SH = S // 2          # query rows per core
P = 128
NK = S // P          # 32 k-tiles
NC = DIM // P        # 6 contraction tiles for projections
QF = 512             # matmul moving free dim
CF = 1024            # projection chunk width
NQC = SH // QF       # 4 q-chunks per core
NKC = S // CF        # 4 K/V projection chunks
KPC = CF // P        # 8 k-tiles per projection chunk
SCALE = 1.0 / np.sqrt(np.float32(D))
GROUPS = [(0, 8), (8, 16), (16, 24), (24, 28), (28, 32)]
RED_G = 4            # the redundant (no-AR) group index
ATTN_FROM = [17, 24, 29, 99, 0]  # subslot from which group may drain

REPLICA_GROUPS = [[0, 1], [2, 3], [4, 5], [6, 7]]

BF16 = mybir.dt.bfloat16
F32 = mybir.dt.float32


def _group_of(kt):
    return kt // 8 if kt < 24 else 3 + (kt - 24) // 4


def _build():
    nc = bacc.Bacc(
        "TRN2",
        target_bir_lowering=False,
        debug=False,
        num_devices=N_CORES,
    )

    # x inputs are pre-tiled on the host to [partition, chunk, ci, col] so
    # each 1024-wide chunk DMA is one contiguous 12KB run per partition
    # (6x fewer descriptors than the strided (i p) f -> p i f view).
    xt_d = nc.dram_tensor("xt", [P, NKC, NC, CF], BF16, kind="ExternalInput")
    xtq_d = nc.dram_tensor("xtq", [P, 2, NC, CF], BF16, kind="ExternalInput")
    xtq2_d = nc.dram_tensor("xtq2", [P, 2, NC, CF], BF16, kind="ExternalInput")
    wq_d = nc.dram_tensor("wq", [DIM, 112], BF16, kind="ExternalInput")
    wkv_d = nc.dram_tensor("wkv", [DIM, 112], BF16, kind="ExternalInput")
    bq_d = nc.dram_tensor("bq", [112, 1], F32, kind="ExternalInput")
    bkv_d = nc.dram_tensor("bkv", [112, 1], F32, kind="ExternalInput")
    out_d = nc.dram_tensor("out", [D, SH], F32, kind="ExternalOutput")

    with tile.TileContext(nc) as tc:
        with (
            tc.tile_pool(name="consts", bufs=1) as consts,
            tc.tile_pool(name="big", bufs=1) as big,
            tc.tile_pool(name="xtp", bufs=3) as xtp,
            tc.tile_pool(name="ps", bufs=3, space="PSUM") as ps,
            tc.tile_pool(name="po", bufs=1, space="PSUM") as pop,
            tc.tile_pool(name="dram", bufs=1, space="DRAM") as dram,
        ):
            # ---- collective stream pre-warm (absorbs first-CC setup) ------
            warm_sb = consts.tile([P, 2], F32, tag="warm")
            nc.vector.memset(warm_sb, 0.0)
            warm_in = dram.tile([P, 2], F32, tag="warm_in")
            warm_out = dram.tile([P, 2], F32, tag="warm_out")
            nc.sync.dma_start(out=warm_in, in_=warm_sb)
            nc.gpsimd.collective_compute(
                "AllReduce",
                mybir.AluOpType.add,
                replica_groups=REPLICA_GROUPS,
                ins=[warm_in.opt()],
                outs=[warm_out.opt()],
            )

            # ---- constants (Q path first so the exp phase starts early) ---
            wq_sb = consts.tile([P, NC, 112], BF16, tag="wq")
            nc.sync.dma_start(out=wq_sb, in_=wq_d[:, :].rearrange("(i p) d -> p i d", p=P))

            # ---- persistent SBUF tensors ----------------------------------
            kT_sb = big.tile([112, S], BF16, tag="kT")
            vT_sb = big.tile([112, S], BF16, tag="vT")
            qT_sb = big.tile([112, SH], BF16, tag="qT")
            qT2_sb = big.tile([112, SH], BF16, tag="qT2")  # partner half
            v_sb = big.tile([P, NK, D], BF16, tag="v")
            vs_sb = big.tile([P, NK, D], BF16, tag="vs")
            e_sb = big.tile([P, NK, SH], BF16, tag="e")
            e_junk = big.tile([P, 2, QF], BF16, tag="e_junk")
            colsum = big.tile([P, NK], F32, tag="colsum")
            colsumh = big.tile([P, NK, 2], F32, tag="colsumh")
            colsumh2 = big.tile([P, 4, 2], F32, tag="colsumh2")
            recip = big.tile([P, NK], F32, tag="recip")
            cs_all = big.tile([P, NK], F32, tag="cs_all")
            out_sb = big.tile([D, SH], F32, tag="out")

            # attention accumulator: qc even -> partitions 0-47 of bank
            # qc//2, qc odd -> partitions 64-111. Complete sums per qc.
            po = pop.tile([P, 2, QF], F32, tag="po")

            # ---- Q^T projections (own + partner), Wq packed [Q|0|Q] -------
            qstate = {}

            def q_dma(key, src_d, c):
                xq_t = xtp.tile([P, NC, CF], BF16, tag="xt")
                nc.sync.dma_start(out=xq_t, in_=src_d[:, c, :, :])
                qstate[key] = [xq_t]

            def q_mms_half(key, dst_sb, c, half):
                st = qstate[key]
                if half == 0:
                    pq_t = ps.tile([P, 2, QF], F32, tag="ps")
                    st.append(pq_t)
                xq_t, pq = st[0], st[1]
                hsl = slice(half * QF, (half + 1) * QF)
                for ci in range(NC):
                    nc.tensor.matmul(
                        pq[0:112, half, :], wq_sb[:, ci, :], xq_t[:, ci, hsl],
                        start=(ci == 0), stop=(ci == NC - 1),
                        skip_group_check=True,
                    )
                nc.vector.tensor_scalar(
                    out=dst_sb[0:112, c * CF + half * QF:c * CF + (half + 1) * QF],
                    in0=pq[0:112, half, :], scalar1=bq_sb, scalar2=None,
                    op0=mybir.AluOpType.add,
                )
                if half == 1:
                    qstate.pop(key)

            # ---- K/V projection: 4 chunks of 1024, small pieces -----------
            kvstate = {}

            def kv_piece(c, piece):
                sl = slice(c * CF, (c + 1) * CF)
                if piece == 0:
                    xt_t = xtp.tile([P, NC, CF], BF16, tag="xt")
                    nc.sync.dma_start(out=xt_t, in_=xt_d[:, c, :, :])
                    kvstate[c] = [xt_t, None]
                elif piece in (1, 2, 3, 4):
                    st = kvstate[c]
                    if piece == 1:
                        pkv_t = ps.tile([P, 2, QF], F32, tag="ps")
                        st[1] = pkv_t
                    xt_t, pkv = st
                    half, lo = divmod(piece - 1, 2)
                    hsl = slice(half * QF, (half + 1) * QF)
                    for ci in range(3 * lo, 3 * lo + 3):
                        nc.tensor.matmul(
                            pkv[0:112, half, :], wkv_sb[:, ci, :], xt_t[:, ci, hsl],
                            start=(ci == 0), stop=(ci == NC - 1),
                            skip_group_check=True,
                        )
                elif piece == 5:
                    _, pkv = kvstate[c]
                    nc.vector.tensor_scalar(
                        out=kT_sb[0:D, sl].rearrange("d (a f) -> d a f", f=QF),
                        in0=pkv[0:D, :, :], scalar1=bkv_sb[0:D, :],
                        scalar2=None, op0=mybir.AluOpType.add,
                    )
                    nc.vector.tensor_scalar(
                        out=vT_sb[64:64 + D, sl].rearrange("d (a f) -> d a f", f=QF),
                        in0=pkv[64:64 + D, :, :], scalar1=bkv_sb[64:64 + D, :],
                        scalar2=None, op0=mybir.AluOpType.add,
                    )
                    # duplicate K^T at partitions 64-111 for row-packing
                    nc.sync.dma_start(out=kT_sb[64:64 + D, sl], in_=kT_sb[0:D, sl])
                    kvstate.pop(c, None)
                else:
                    base = c * KPC + (0 if piece == 6 else 4)
                    for kt in range(base, base + 4):
                        pt = ps.tile([P, D], BF16, tag="ps")
                        nc.tensor.transpose(
                            pt, vT_sb[64:64 + D, kt * P:(kt + 1) * P],
                            ident[64:64 + D, 64:64 + D],
                            tile_position=(64, 0),
                        )
                        nc.vector.tensor_copy(out=v_sb[:, kt, :], in_=pt)

            # ---- attention matmuls for one k-tile (col-packed by qc) ------
            # start/stop follow EMISSION order (the tail emits the no-AR
            # group before group 3), tracked by a counter.
            attn_count = [0]

            def emit_attn(kt):
                first = attn_count[0] == 0
                last = attn_count[0] == NK - 1
                attn_count[0] += 1
                for qc in range(NQC):
                    if qc % 2 == 0:
                        reg = po[0:D, qc // 2, :]
                        pos = (0, 0)
                    else:
                        reg = po[64:64 + D, qc // 2, :]
                        pos = (0, 64)
                    nc.tensor.matmul(
                        reg,
                        vs_sb[:, kt, :],
                        e_sb[:, kt, qc * QF:(qc + 1) * QF],
                        start=first, stop=last,
                        tile_position=pos, skip_group_check=True,
                    )

            # ---- startup ---------------------------------------------------
            q_dma("q0", xtq_d, 0)
            bq_sb = consts.tile([112, 1], F32, tag="bq")
            nc.sync.dma_start(out=bq_sb, in_=bq_d[:, :])
            bkv_sb = consts.tile([112, 1], F32, tag="bkv")
            nc.sync.dma_start(out=bkv_sb, in_=bkv_d[:, :])
            wkv_sb = consts.tile([P, NC, 112], BF16, tag="wkv")
            nc.sync.dma_start(out=wkv_sb, in_=wkv_d[:, :].rearrange("(i p) d -> p i d", p=P))
            ident = consts.tile([P, P], BF16, tag="ident")
            make_identity(nc, ident)
            kv_piece(0, 0)
            q_dma("q1", xtq_d, 1)
            # HAM warm-up: ~40 junk matmuls on the resident Wq tile while the
            # first x chunks stream in, so the startup projection chain runs
            # at the warm PE clock instead of 1.2 GHz.
            wub = ps.tile([P, 2, QF], F32, tag="ps")
            wq_flat = wq_sb.rearrange("p i d -> p (i d)")
            for i in range(40):
                nc.tensor.matmul(
                    wub[0:112, i % 2, :], wq_sb[:, i % NC, :],
                    wq_flat[:, 0:QF],
                    start=True, stop=True, skip_group_check=True,
                )
            q_mms_half("q0", qT_sb, 0, 0)
            q_mms_half("q0", qT_sb, 0, 1)
            # pieces 6-7 (V transposes) defer to the filler queue: their
            # outputs are first needed only after the group-0 AllReduce
            for piece in range(1, 6):
                kv_piece(0, piece)
            for c in range(1, NKC):
                kv_piece(c, 0)  # prefetch the remaining chunk DMAs
            q_dma("p0", xtq2_d, 0)
            q_dma("p1", xtq2_d, 1)

            # filler work woven into the scores stream, one small item per
            # subslot (<=6-matmul lumps so the exp stream never starves)
            filler = [("q1", qT_sb, 1, 0), ("q1", qT_sb, 1, 1),
                      ("kv", 0, 6), ("kv", 0, 7)]
            for c in range(1, NKC):
                for piece in range(1, 8):
                    filler.append(("kv", c, piece))
            filler += [("p0", qT2_sb, 0, 0), ("p0", qT2_sb, 0, 1),
                       ("p1", qT2_sb, 1, 0), ("p1", qT2_sb, 1, 1)]

            attn_pending = []

            def drain_attn(subslot, budget):
                for _ in range(budget):
                    if not attn_pending:
                        return
                    if subslot < ATTN_FROM[_group_of(attn_pending[0])]:
                        return
                    emit_attn(attn_pending.pop(0))

            # ---- main pipeline: per group, h-major sweeps -----------------
            subslot = 0
            for g, (kt_lo, kt_hi) in enumerate(GROUPS):
                pairs = [(kt, kt + 1) for kt in range(kt_lo, kt_hi, 2)]
                red = (g == RED_G)
                for h in range(2):
                    for ktA, ktB in pairs:
                        sctA = ps.tile([P, 2, QF], F32, tag="ps")
                        sctB = ps.tile([P, 2, QF], F32, tag="ps")
                        for qh in range(2):
                            qc = 2 * h + qh
                            qsl = slice(qc * QF, (qc + 1) * QF)
                            # row-packed pair: ktA on array rows 0-47, ktB
                            # on rows 64-111 -> the two streams overlap.
                            nc.tensor.matmul(
                                sctA[:, qh, :],
                                kT_sb[0:D, ktA * P:(ktA + 1) * P],
                                qT_sb[0:D, qsl],
                                start=True, stop=True,
                            )
                            nc.tensor.matmul(
                                sctB[:, qh, :],
                                kT_sb[64:64 + D, ktB * P:(ktB + 1) * P],
                                qT_sb[64:64 + D, qsl],
                                start=True, stop=True,
                            )
                        for sct, kt in ((sctA, ktA), (sctB, ktB)):
                            nc.scalar.activation(
                                out=e_sb[:, kt, h * 2 * QF:(h + 1) * 2 * QF],
                                in_=sct[:, :, :],
                                func=mybir.ActivationFunctionType.Exp,
                                scale=float(SCALE),
                                accum_out=colsumh[:, kt, h:h + 1],
                            )
                        if red:
                            # partner-half scores/exp for the same k-tiles:
                            # only the accumulated colsum is kept, so this
                            # group needs no AllReduce.
                            sct2A = ps.tile([P, 2, QF], F32, tag="ps")
                            sct2B = ps.tile([P, 2, QF], F32, tag="ps")
                            for qh in range(2):
                                qsl = slice((2 * h + qh) * QF, (2 * h + qh + 1) * QF)
                                nc.tensor.matmul(
                                    sct2A[:, qh, :],
                                    kT_sb[0:D, ktA * P:(ktA + 1) * P],
                                    qT2_sb[0:D, qsl],
                                    start=True, stop=True,
                                )
                                nc.tensor.matmul(
                                    sct2B[:, qh, :],
                                    kT_sb[64:64 + D, ktB * P:(ktB + 1) * P],
                                    qT2_sb[64:64 + D, qsl],
                                    start=True, stop=True,
                                )
                            for sct2, kt in ((sct2A, ktA), (sct2B, ktB)):
                                nc.scalar.activation(
                                    out=e_junk[:, :, :],
                                    in_=sct2[:, :, :],
                                    func=mybir.ActivationFunctionType.Exp,
                                    scale=float(SCALE),
                                    accum_out=colsumh2[:, kt - 28, h:h + 1],
                                )
                        if filler:
                            item = filler.pop(0)
                            if item[0] == "kv":
                                kv_piece(item[1], item[2])
                            else:
                                q_mms_half(item[0], item[1], item[2], item[3])
                        if h == 1:
                            for kt in (ktA, ktB):
                                nc.vector.tensor_add(
                                    out=colsum[:, kt:kt + 1],
                                    in0=colsumh[:, kt, 0:1],
                                    in1=colsumh[:, kt, 1:2],
                                )
                                if red:
                                    nc.vector.tensor_add(
                                        out=colsum[:, kt:kt + 1],
                                        in0=colsum[:, kt:kt + 1],
                                        in1=colsumh2[:, kt - 28, 0:1],
                                    )
                                    nc.vector.tensor_add(
                                        out=colsum[:, kt:kt + 1],
                                        in0=colsum[:, kt:kt + 1],
                                        in1=colsumh2[:, kt - 28, 1:2],
                                    )
                                    nc.vector.reciprocal(
                                        out=recip[:, kt:kt + 1],
                                        in_=colsum[:, kt:kt + 1],
                                    )
                                    nc.vector.tensor_scalar(
                                        out=vs_sb[:, kt, :],
                                        in0=v_sb[:, kt, :],
                                        scalar1=recip[:, kt:kt + 1],
                                        scalar2=None,
                                        op0=mybir.AluOpType.mult,
                                    )
                                    attn_pending.append(kt)
                        drain_attn(subslot, 2)
                        subslot += 1

                if red:
                    continue
                # issue the group's colsum AllReduce as soon as its exps done
                gsl = slice(kt_lo, kt_hi)
                gn = kt_hi - kt_lo
                cs_in = dram.tile([P, gn], F32, tag=f"cs_in{g}")
                cs_out = dram.tile([P, gn], F32, tag=f"cs_out{g}")
                nc.sync.dma_start(out=cs_in, in_=colsum[:, gsl])
                nc.gpsimd.collective_compute(
                    "AllReduce",
                    mybir.AluOpType.add,
                    replica_groups=REPLICA_GROUPS,
                    ins=[cs_in.opt()],
                    outs=[cs_out.opt()],
                )
                nc.sync.dma_start(out=cs_all[:, gsl], in_=cs_out)
                nc.vector.reciprocal(out=recip[:, gsl], in_=cs_all[:, gsl])
                for kt in range(kt_lo, kt_hi):
                    nc.vector.tensor_scalar(
                        out=vs_sb[:, kt, :],
                        in0=v_sb[:, kt, :],
                        scalar1=recip[:, kt:kt + 1], scalar2=None,
                        op0=mybir.AluOpType.mult,
                    )
                    attn_pending.append(kt)

            # ---- tail: drain remaining attention (g4 first: no AR) --------
            attn_pending.sort(key=lambda kt: (_group_of(kt) != RED_G, kt))
            ordered = attn_pending[:]
            attn_pending.clear()
            for kt in ordered:
                emit_attn(kt)

            out3 = out_sb.rearrange("d (c f) -> d c f", f=QF)
            for qc in range(NQC):
                src = po[0:D, qc // 2, :] if qc % 2 == 0 else po[64:64 + D, qc // 2, :]
                nc.vector.tensor_copy(out=out3[:, qc, :], in_=src)
                nc.sync.dma_start(
                    out=out_d[:, qc * QF:(qc + 1) * QF], in_=out3[:, qc, :]
                )

    nc.compile()
    return nc


_NC_CACHE = None


def _get_nc():
    global _NC_CACHE
    if _NC_CACHE is None:
        _NC_CACHE = _build()
    return _NC_CACHE


def kernel(x, Wq, bq, Wk, bk, Wv, bv):
    x = np.asarray(x, np.float32)
    bf = ml_dtypes.bfloat16
    wq2 = np.zeros((DIM, 112), np.float32)
    wq2[:, 0:D] = np.asarray(Wq, np.float32)
    wq2[:, 64:64 + D] = np.asarray(Wq, np.float32)
    bq2 = np.zeros((112,), np.float32)
    bq2[0:D] = np.asarray(bq, np.float32).ravel()
    bq2[64:64 + D] = np.asarray(bq, np.float32).ravel()
    wkv = np.zeros((DIM, 112), np.float32)
    wkv[:, 0:D] = np.asarray(Wk, np.float32)
    wkv[:, 64:64 + D] = np.asarray(Wv, np.float32)
    bkv = np.zeros((112,), np.float32)
    bkv[0:D] = np.asarray(bk, np.float32).ravel()
    bkv[64:64 + D] = np.asarray(bv, np.float32).ravel()
    w_bf = {
        "wq": np.ascontiguousarray(wq2).astype(bf),
        "wkv": np.ascontiguousarray(wkv).astype(bf),
    }
    b_f32 = {
        "bq": np.ascontiguousarray(bq2).reshape(112, 1),
        "bkv": np.ascontiguousarray(bkv).reshape(112, 1),
    }

    def tile_x(xt_full, n_chunks):
        # [768, n_chunks*1024] -> [128, n_chunks, 6, 1024] (d = ci*128 + p)
        a = xt_full.reshape(NC, P, n_chunks, CF)
        return np.ascontiguousarray(a.transpose(1, 2, 0, 3))

    in_maps = []
    for core in range(N_CORES):
        b_idx, h = divmod(core, 2)
        xt = np.ascontiguousarray(x[b_idx].T).astype(bf)          # [768, 4096]
        xtq = tile_x(xt[:, h * SH:(h + 1) * SH], 2)               # own half
        xtq2 = tile_x(xt[:, (1 - h) * SH:(2 - h) * SH], 2)        # partner
        in_maps.append({"xt": tile_x(xt, NKC), "xtq": xtq, "xtq2": xtq2,
                        **w_bf, **b_f32})

    res = run_bass_kernel_spmd(
        _get_nc(), in_maps, core_ids=list(range(N_CORES)), trace=False
    )

    out = np.empty((B, S, D), np.float32)
    for core in range(N_CORES):
        b_idx, h = divmod(core, 2)
        out[b_idx, h * SH:(h + 1) * SH, :] = res.results[core]["out"].T
    return out


# revision 42
# speedup vs baseline: 1.1714x; 1.1714x over previous
"""Distributed Trainium2 Bass kernel for single-head attention with
softmax over the QUERY axis (faithful to the reference).

Reference math (per batch b):
    q = x @ Wq + bq          # [S, D]   S=4096, D=48
    k = x @ Wk + bk
    v = x @ Wv + bv
    s = (q @ k.T) / sqrt(D)  # [S_q, S_k]
    p = softmax(s, axis=QUERY)          # normalize each k-COLUMN over q
    out = p @ v              # [S_q, D]

Sharding: 8 cores = 4 batches x 2 query-halves. Core c handles batch
c//2, query rows [ (c%2)*2048, (c%2+1)*2048 ).

Layout: everything is computed TRANSPOSED on chip; scores_T[k, q] tiles
have k on partitions / q on the free axis so the softmax denominator
colsum[k] = sum_q exp(s[q,k]) falls out of the ScalarEngine's
activation accumulator, and the normalization folds into V. Output is
out_T [48, 2048], transposed on host.

Structure (the kernel is ScalarEngine/exp-bound; collectives on this
stack are high-variance, so the design minimizes exposure to them):
  - PE: scores matmuls 2x ROW-packed (K^T/Q^T duplicated at partitions
    64-111 via a 112-col packed Wq and an SBUF->SBUF DMA; two k-tiles
    stream concurrently in disjoint row-quadrants), attention matmuls
    2x COLUMN-packed by qc parity into a persistent 2-bank PSUM
    accumulator, projections in 1024-wide chunks woven as small filler
    pieces so the exp stream never starves.
  - Scores run in h-major sweeps per k-tile group, so the second Q
    chunk's projection hides inside the first sweep.
  - Colsum AllReduces: a dummy AR at t=0 absorbs stream setup; groups
    [0:8/8:16/16:24/24:28] AR mid-phase while exps continue. The LAST
    group (k-tiles 28-31) needs NO collective: both pair cores compute
    those 512 columns' partner-half exp sums redundantly (the full x
    is resident; +8 scratch ACTIVATEs) so its colsum is local and its
    attention + the output epilogue form a short deterministic tail.
    The redundant sweep also gives the group-3 AR ~25us of slack.

exp() runs without max-subtraction: scores*scale is N(0,~1/9), bounded
by ~|2.5| for these inputs, so exp stays well inside fp32 range
(softmax is shift-invariant, so the result matches the reference).
"""

import sys

for _p in ("/opt/trn_rl_repo",):
    if _p not in sys.path:
        sys.path.insert(0, _p)

import numpy as np
import ml_dtypes

import concourse.bass as bass
import concourse.tile as tile
from concourse import bacc, mybir
from concourse.bass_utils import run_bass_kernel_spmd
from concourse.masks import make_identity

N_CORES = 8
B = 4
S = 4096
DIM = 768
D = 48
SH = S // 2          # query rows per core
P = 128
NK = S // P          # 32 k-tiles
NC = DIM // P        # 6 contraction tiles for projections
QF = 512             # matmul moving free dim
CF = 1024            # projection chunk width
NQC = SH // QF       # 4 q-chunks per core
NKC = S // CF        # 4 K/V projection chunks
KPC = CF // P        # 8 k-tiles per projection chunk
SCALE = 1.0 / np.sqrt(np.float32(D))
GROUPS = [(0, 8), (8, 16), (16, 24), (24, 28), (28, 32)]
RED_G = 4            # the redundant (no-AR) group index
ATTN_FROM = [17, 24, 29, 99, 0]  # subslot from which group may drain

REPLICA_GROUPS = [[0, 1], [2, 3], [4, 5], [6, 7]]

BF16 = mybir.dt.bfloat16
F32 = mybir.dt.float32


def _group_of(kt):
    return kt // 8 if kt < 24 else 3 + (kt - 24) // 4


def _build():
    nc = bacc.Bacc(
        "TRN2",
        target_bir_lowering=False,
        debug=False,
        num_devices=N_CORES,
    )

    # x inputs are pre-tiled on the host to [partition, chunk, ci, col] so
    # each 1024-wide chunk DMA is one contiguous 12KB run per partition
    # (6x fewer descriptors than the strided (i p) f -> p i f view).
    xt_d = nc.dram_tensor("xt", [P, NKC, NC, CF], BF16, kind="ExternalInput")
    xtq_d = nc.dram_tensor("xtq", [P, 2, NC, CF], BF16, kind="ExternalInput")
    xtq2_d = nc.dram_tensor("xtq2", [P, 2, NC, CF], BF16, kind="ExternalInput")
    wq_d = nc.dram_tensor("wq", [DIM, 112], BF16, kind="ExternalInput")
    wkv_d = nc.dram_tensor("wkv", [DIM, 112], BF16, kind="ExternalInput")
    bq_d = nc.dram_tensor("bq", [112, 1], F32, kind="ExternalInput")
    bkv_d = nc.dram_tensor("bkv", [112, 1], F32, kind="ExternalInput")
    out_d = nc.dram_tensor("out", [D, SH], F32, kind="ExternalOutput")

    with tile.TileContext(nc) as tc:
        with (
            tc.tile_pool(name="consts", bufs=1) as consts,
            tc.tile_pool(name="big", bufs=1) as big,
            tc.tile_pool(name="xtp", bufs=3) as xtp,
            tc.tile_pool(name="ps", bufs=3, space="PSUM") as ps,
            tc.tile_pool(name="po", bufs=1, space="PSUM") as pop,
            tc.tile_pool(name="dram", bufs=1, space="DRAM") as dram,
        ):
            # ---- collective stream pre-warm (absorbs first-CC setup) ------
            warm_sb = consts.tile([P, 2], F32, tag="warm")
            nc.vector.memset(warm_sb, 0.0)
            warm_in = dram.tile([P, 2], F32, tag="warm_in")
            warm_out = dram.tile([P, 2], F32, tag="warm_out")
            nc.sync.dma_start(out=warm_in, in_=warm_sb)
            nc.gpsimd.collective_compute(
                "AllReduce",
                mybir.AluOpType.add,
                replica_groups=REPLICA_GROUPS,
                ins=[warm_in.opt()],
                outs=[warm_out.opt()],
            )

            # ---- constants (Q path first so the exp phase starts early) ---
            wq_sb = consts.tile([P, NC, 112], BF16, tag="wq")
            nc.sync.dma_start(out=wq_sb, in_=wq_d[:, :].rearrange("(i p) d -> p i d", p=P))

            # ---- persistent SBUF tensors ----------------------------------
            kT_sb = big.tile([112, S], BF16, tag="kT")
            vT_sb = big.tile([112, S], BF16, tag="vT")
            qT_sb = big.tile([112, SH], BF16, tag="qT")
            qT2_sb = big.tile([112, SH], BF16, tag="qT2")  # partner half
            v_sb = big.tile([P, NK, D], BF16, tag="v")
            vs_sb = big.tile([P, NK, D], BF16, tag="vs")
            e_sb = big.tile([P, NK, SH], BF16, tag="e")
            e_junk = big.tile([P, 2, QF], BF16, tag="e_junk")
            colsum = big.tile([P, NK], F32, tag="colsum")
            colsumh = big.tile([P, NK, 2], F32, tag="colsumh")
            colsumh2 = big.tile([P, 4, 2], F32, tag="colsumh2")
            recip = big.tile([P, NK], F32, tag="recip")
            cs_all = big.tile([P, NK], F32, tag="cs_all")
            out_sb = big.tile([D, SH], F32, tag="out")

            # attention accumulator: qc even -> partitions 0-47 of bank
            # qc//2, qc odd -> partitions 64-111. Complete sums per qc.
            po = pop.tile([P, 2, QF], F32, tag="po")

            # ---- Q^T projections (own + partner), Wq packed [Q|0|Q] -------
            qstate = {}

            def q_dma(key, src_d, c):
                xq_t = xtp.tile([P, NC, CF], BF16, tag="xt")
                nc.sync.dma_start(out=xq_t, in_=src_d[:, c, :, :])
                qstate[key] = [xq_t]

            def q_mms_half(key, dst_sb, c, half):
                st = qstate[key]
                if half == 0:
                    pq_t = ps.tile([P, 2, QF], F32, tag="ps")
                    st.append(pq_t)
                xq_t, pq = st[0], st[1]
                hsl = slice(half * QF, (half + 1) * QF)
                for ci in range(NC):
                    nc.tensor.matmul(
                        pq[0:112, half, :], wq_sb[:, ci, :], xq_t[:, ci, hsl],
                        start=(ci == 0), stop=(ci == NC - 1),
                        skip_group_check=True,
                    )
                nc.vector.tensor_scalar(
                    out=dst_sb[0:112, c * CF + half * QF:c * CF + (half + 1) * QF],
                    in0=pq[0:112, half, :], scalar1=bq_sb, scalar2=None,
                    op0=mybir.AluOpType.add,
                )
                if half == 1:
                    qstate.pop(key)

            # ---- K/V projection: 4 chunks of 1024, small pieces -----------
            kvstate = {}

            def kv_piece(c, piece):
                sl = slice(c * CF, (c + 1) * CF)
                if piece == 0:
                    xt_t = xtp.tile([P, NC, CF], BF16, tag="xt")
                    nc.sync.dma_start(out=xt_t, in_=xt_d[:, c, :, :])
                    kvstate[c] = [xt_t, None]
                elif piece in (1, 2, 3, 4):
                    st = kvstate[c]
                    if piece == 1:
                        pkv_t = ps.tile([P, 2, QF], F32, tag="ps")
                        st[1] = pkv_t
                    xt_t, pkv = st
                    half, lo = divmod(piece - 1, 2)
                    hsl = slice(half * QF, (half + 1) * QF)
                    for ci in range(3 * lo, 3 * lo + 3):
                        nc.tensor.matmul(
                            pkv[0:112, half, :], wkv_sb[:, ci, :], xt_t[:, ci, hsl],
                            start=(ci == 0), stop=(ci == NC - 1),
                            skip_group_check=True,
                        )
                elif piece == 5:
                    _, pkv = kvstate[c]
                    nc.vector.tensor_scalar(
                        out=kT_sb[0:D, sl].rearrange("d (a f) -> d a f", f=QF),
                        in0=pkv[0:D, :, :], scalar1=bkv_sb[0:D, :],
                        scalar2=None, op0=mybir.AluOpType.add,
                    )
                    nc.vector.tensor_scalar(
                        out=vT_sb[64:64 + D, sl].rearrange("d (a f) -> d a f", f=QF),
                        in0=pkv[64:64 + D, :, :], scalar1=bkv_sb[64:64 + D, :],
                        scalar2=None, op0=mybir.AluOpType.add,
                    )
                    # duplicate K^T at partitions 64-111 for row-packing
                    nc.sync.dma_start(out=kT_sb[64:64 + D, sl], in_=kT_sb[0:D, sl])
                    kvstate.pop(c, None)
                else:
                    base = c * KPC + (0 if piece == 6 else 4)
                    for kt in range(base, base + 4):
                        pt = ps.tile([P, D], BF16, tag="ps")
                        nc.tensor.transpose(
                            pt, vT_sb[64:64 + D, kt * P:(kt + 1) * P],
                            ident[64:64 + D, 64:64 + D],
                            tile_position=(64, 0),
                        )
                        nc.vector.tensor_copy(out=v_sb[:, kt, :], in_=pt)

            # ---- attention matmuls for one k-tile (col-packed by qc) ------
            # start/stop follow EMISSION order (the tail emits the no-AR
            # group before group 3), tracked by a counter.
            attn_count = [0]

            def emit_attn(kt):
                first = attn_count[0] == 0
                last = attn_count[0] == NK - 1
                attn_count[0] += 1
                for qc in range(NQC):
                    if qc % 2 == 0:
                        reg = po[0:D, qc // 2, :]
                        pos = (0, 0)
                    else:
                        reg = po[64:64 + D, qc // 2, :]
                        pos = (0, 64)
                    nc.tensor.matmul(
                        reg,
                        vs_sb[:, kt, :],
                        e_sb[:, kt, qc * QF:(qc + 1) * QF],
                        start=first, stop=last,
                        tile_position=pos, skip_group_check=True,
                    )

            # ---- startup ---------------------------------------------------
            q_dma("q0", xtq_d, 0)
            bq_sb = consts.tile([112, 1], F32, tag="bq")
            nc.sync.dma_start(out=bq_sb, in_=bq_d[:, :])
            bkv_sb = consts.tile([112, 1], F32, tag="bkv")
            nc.sync.dma_start(out=bkv_sb, in_=bkv_d[:, :])
            wkv_sb = consts.tile([P, NC, 112], BF16, tag="wkv")
            nc.sync.dma_start(out=wkv_sb, in_=wkv_d[:, :].rearrange("(i p) d -> p i d", p=P))
            ident = consts.tile([P, P], BF16, tag="ident")
            make_identity(nc, ident)
            kv_piece(0, 0)
            q_dma("q1", xtq_d, 1)
            # HAM warm-up: ~40 junk matmuls on the resident Wq tile while the
            # first x chunks stream in, so the startup projection chain runs
            # at the warm PE clock instead of 1.2 GHz.
            wub = ps.tile([P, 2, QF], F32, tag="ps")
            wq_flat = wq_sb.rearrange("p i d -> p (i d)")
            for i in range(40):
                nc.tensor.matmul(
                    wub[0:112, i % 2, :], wq_sb[:, i % NC, :],
                    wq_flat[:, 0:QF],
                    start=True, stop=True, skip_group_check=True,
                )
            q_mms_half("q0", qT_sb, 0, 0)
            q_mms_half("q0", qT_sb, 0, 1)
            for piece in range(1, 8):
                kv_piece(0, piece)
            for c in range(1, NKC):
                kv_piece(c, 0)  # prefetch the remaining chunk DMAs
            q_dma("p0", xtq2_d, 0)
            q_dma("p1", xtq2_d, 1)

            # filler work woven into the scores stream, one small item per
            # subslot (<=6-matmul lumps so the exp stream never starves)
            filler = [("q1", qT_sb, 1, 0), ("q1", qT_sb, 1, 1)]
            for c in range(1, NKC):
                for piece in range(1, 8):
                    filler.append(("kv", c, piece))
            filler += [("p0", qT2_sb, 0, 0), ("p0", qT2_sb, 0, 1),
                       ("p1", qT2_sb, 1, 0), ("p1", qT2_sb, 1, 1)]

            attn_pending = []

            def drain_attn(subslot, budget):
                for _ in range(budget):
                    if not attn_pending:
                        return
                    if subslot < ATTN_FROM[_group_of(attn_pending[0])]:
                        return
                    emit_attn(attn_pending.pop(0))

            # ---- main pipeline: per group, h-major sweeps -----------------
            subslot = 0
            for g, (kt_lo, kt_hi) in enumerate(GROUPS):
                pairs = [(kt, kt + 1) for kt in range(kt_lo, kt_hi, 2)]
                red = (g == RED_G)
                for h in range(2):
                    for ktA, ktB in pairs:
                        sctA = ps.tile([P, 2, QF], F32, tag="ps")
                        sctB = ps.tile([P, 2, QF], F32, tag="ps")
                        for qh in range(2):
                            qc = 2 * h + qh
                            qsl = slice(qc * QF, (qc + 1) * QF)
                            # row-packed pair: ktA on array rows 0-47, ktB
                            # on rows 64-111 -> the two streams overlap.
                            nc.tensor.matmul(
                                sctA[:, qh, :],
                                kT_sb[0:D, ktA * P:(ktA + 1) * P],
                                qT_sb[0:D, qsl],
                                start=True, stop=True,
                            )
                            nc.tensor.matmul(
                                sctB[:, qh, :],
                                kT_sb[64:64 + D, ktB * P:(ktB + 1) * P],
                                qT_sb[64:64 + D, qsl],
                                start=True, stop=True,
                            )
                        for sct, kt in ((sctA, ktA), (sctB, ktB)):
                            nc.scalar.activation(
                                out=e_sb[:, kt, h * 2 * QF:(h + 1) * 2 * QF],
                                in_=sct[:, :, :],
                                func=mybir.ActivationFunctionType.Exp,
                                scale=float(SCALE),
                                accum_out=colsumh[:, kt, h:h + 1],
                            )
                        if red:
                            # partner-half scores/exp for the same k-tiles:
                            # only the accumulated colsum is kept, so this
                            # group needs no AllReduce.
                            sct2A = ps.tile([P, 2, QF], F32, tag="ps")
                            sct2B = ps.tile([P, 2, QF], F32, tag="ps")
                            for qh in range(2):
                                qsl = slice((2 * h + qh) * QF, (2 * h + qh + 1) * QF)
                                nc.tensor.matmul(
                                    sct2A[:, qh, :],
                                    kT_sb[0:D, ktA * P:(ktA + 1) * P],
                                    qT2_sb[0:D, qsl],
                                    start=True, stop=True,
                                )
                                nc.tensor.matmul(
                                    sct2B[:, qh, :],
                                    kT_sb[64:64 + D, ktB * P:(ktB + 1) * P],
                                    qT2_sb[64:64 + D, qsl],
                                    start=True, stop=True,
                                )
                            for sct2, kt in ((sct2A, ktA), (sct2B, ktB)):
                                nc.scalar.activation(
                                    out=e_junk[:, :, :],
                                    in_=sct2[:, :, :],
                                    func=mybir.ActivationFunctionType.Exp,
                                    scale=float(SCALE),
                                    accum_out=colsumh2[:, kt - 28, h:h + 1],
                                )
                        if filler:
                            item = filler.pop(0)
                            if item[0] == "kv":
                                kv_piece(item[1], item[2])
                            else:
                                q_mms_half(item[0], item[1], item[2], item[3])
                        if h == 1:
                            for kt in (ktA, ktB):
                                nc.vector.tensor_add(
                                    out=colsum[:, kt:kt + 1],
                                    in0=colsumh[:, kt, 0:1],
                                    in1=colsumh[:, kt, 1:2],
                                )
                                if red:
                                    nc.vector.tensor_add(
                                        out=colsum[:, kt:kt + 1],
                                        in0=colsum[:, kt:kt + 1],
                                        in1=colsumh2[:, kt - 28, 0:1],
                                    )
                                    nc.vector.tensor_add(
                                        out=colsum[:, kt:kt + 1],
                                        in0=colsum[:, kt:kt + 1],
                                        in1=colsumh2[:, kt - 28, 1:2],
                                    )
                                    nc.vector.reciprocal(
                                        out=recip[:, kt:kt + 1],
                                        in_=colsum[:, kt:kt + 1],
                                    )
                                    nc.vector.tensor_scalar(
                                        out=vs_sb[:, kt, :],
                                        in0=v_sb[:, kt, :],
                                        scalar1=recip[:, kt:kt + 1],
                                        scalar2=None,
                                        op0=mybir.AluOpType.mult,
                                    )
                                    attn_pending.append(kt)
                        drain_attn(subslot, 2)
                        subslot += 1

                if red:
                    continue
                # issue the group's colsum AllReduce as soon as its exps done
                gsl = slice(kt_lo, kt_hi)
                gn = kt_hi - kt_lo
                cs_in = dram.tile([P, gn], F32, tag=f"cs_in{g}")
                cs_out = dram.tile([P, gn], F32, tag=f"cs_out{g}")
                nc.sync.dma_start(out=cs_in, in_=colsum[:, gsl])
                nc.gpsimd.collective_compute(
                    "AllReduce",
                    mybir.AluOpType.add,
                    replica_groups=REPLICA_GROUPS,
                    ins=[cs_in.opt()],
                    outs=[cs_out.opt()],
                )
                nc.sync.dma_start(out=cs_all[:, gsl], in_=cs_out)
                nc.vector.reciprocal(out=recip[:, gsl], in_=cs_all[:, gsl])
                for kt in range(kt_lo, kt_hi):
                    nc.vector.tensor_scalar(
                        out=vs_sb[:, kt, :],
                        in0=v_sb[:, kt, :],
                        scalar1=recip[:, kt:kt + 1], scalar2=None,
                        op0=mybir.AluOpType.mult,
                    )
                    attn_pending.append(kt)

            # ---- tail: drain remaining attention (g4 first: no AR) --------
            attn_pending.sort(key=lambda kt: (_group_of(kt) != RED_G, kt))
            ordered = attn_pending[:]
            attn_pending.clear()
            for kt in ordered:
                emit_attn(kt)

            out3 = out_sb.rearrange("d (c f) -> d c f", f=QF)
            for qc in range(NQC):
                src = po[0:D, qc // 2, :] if qc % 2 == 0 else po[64:64 + D, qc // 2, :]
                nc.vector.tensor_copy(out=out3[:, qc, :], in_=src)
                nc.sync.dma_start(
                    out=out_d[:, qc * QF:(qc + 1) * QF], in_=out3[:, qc, :]
                )

    nc.compile()
    return nc


_NC_CACHE = None


def _get_nc():
    global _NC_CACHE
    if _NC_CACHE is None:
        _NC_CACHE = _build()
    return _NC_CACHE


def kernel(x, Wq, bq, Wk, bk, Wv, bv):
    x = np.asarray(x, np.float32)
    bf = ml_dtypes.bfloat16
    wq2 = np.zeros((DIM, 112), np.float32)
    wq2[:, 0:D] = np.asarray(Wq, np.float32)
    wq2[:, 64:64 + D] = np.asarray(Wq, np.float32)
    bq2 = np.zeros((112,), np.float32)
    bq2[0:D] = np.asarray(bq, np.float32).ravel()
    bq2[64:64 + D] = np.asarray(bq, np.float32).ravel()
    wkv = np.zeros((DIM, 112), np.float32)
    wkv[:, 0:D] = np.asarray(Wk, np.float32)
    wkv[:, 64:64 + D] = np.asarray(Wv, np.float32)
    bkv = np.zeros((112,), np.float32)
    bkv[0:D] = np.asarray(bk, np.float32).ravel()
    bkv[64:64 + D] = np.asarray(bv, np.float32).ravel()
    w_bf = {
        "wq": np.ascontiguousarray(wq2).astype(bf),
        "wkv": np.ascontiguousarray(wkv).astype(bf),
    }
    b_f32 = {
        "bq": np.ascontiguousarray(bq2).reshape(112, 1),
        "bkv": np.ascontiguousarray(bkv).reshape(112, 1),
    }

    def tile_x(xt_full, n_chunks):
        # [768, n_chunks*1024] -> [128, n_chunks, 6, 1024] (d = ci*128 + p)
        a = xt_full.reshape(NC, P, n_chunks, CF)
        return np.ascontiguousarray(a.transpose(1, 2, 0, 3))

    in_maps = []
    for core in range(N_CORES):
        b_idx, h = divmod(core, 2)
        xt = np.ascontiguousarray(x[b_idx].T).astype(bf)          # [768, 4096]
        xtq = tile_x(xt[:, h * SH:(h + 1) * SH], 2)               # own half
        xtq2 = tile_x(xt[:, (1 - h) * SH:(2 - h) * SH], 2)        # partner
        in_maps.append({"xt": tile_x(xt, NKC), "xtq": xtq, "xtq2": xtq2,
                        **w_bf, **b_f32})

    res = run_bass_kernel_spmd(
        _get_nc(), in_maps, core_ids=list(range(N_CORES)), trace=False
    )

    out = np.empty((B, S, D), np.float32)
    for core in range(N_CORES):
        b_idx, h = divmod(core, 2)
        out[b_idx, h * SH:(h + 1) * SH, :] = res.results[core]["out"].T
    return out
